# revision 8
# baseline (speedup 1.0000x reference)
"""Trainium2 Bass kernel for the AdaptiveClustering loss.

Computes, for inputs z[B,C,D], kernel_weight[C,D], o[B,C], y[B]:
  loss_cls   = mean((o - one_hot(y))^2)
  loss_close = sum_c segsum_c(||z[i,y_i]-w[y_i]||^2) / (count_c * D)
  loss_dist  = sum_c relu(1 - S_c)/C,  S_c = sum_j ||w_c - w_j||^2 / ((C-1)D)
Returns (total, loss_cls, loss_close, loss_dist).

Strategy: data-parallel over B across 8 cores.  Each core:
  - gathers only the B_loc "own-class" rows z[i, y_i, :] via indirect DMA
    (so only ~0.5MB of the 52MB z shard is ever read),
  - builds a one-hot(y) tile and uses ONE accumulated PE matmul to produce,
    per class c: sum_{i in c} z_own[i,:] (M), sum_{i in c} ||z_own_i||^2,
    and count_c, using  ||z_i - w_c||^2 = ||z_i||^2 - 2 z_i.w_c + ||w_c||^2,
  - reduces o^2 and o[i,y_i] per-partition for loss_cls,
  - computes W@W^T row sums for loss_dist.
Per-core partials ([128,8] f32) are combined on the host (a ~1000-flop
epilogue, the "unshard" step).
"""

import numpy as np

import concourse.bacc as bacc
import concourse.bass as bass
import concourse.mybir as mybir
import concourse.tile as tile
from concourse.bass_utils import run_bass_kernel_spmd

B, C, D = 8192, 100, 128
NCORES = 8
BL = B // NCORES  # 1024 samples per core
P = 128
NCH = BL // P  # 8 free-dim slots per partition (sample i = 8*p + n)
MARGIN = 1.0

F32 = mybir.dt.float32
I32 = mybir.dt.int32
OP = mybir.AluOpType
AF = mybir.ActivationFunctionType


def build_module():
    nc = bacc.Bacc("TRN2", target_bir_lowering=False, debug=False)

    z_d = nc.dram_tensor("z", [BL * C, D], F32, kind="ExternalInput")
    o_d = nc.dram_tensor("o", [P, NCH * C], F32, kind="ExternalInput")
    y_d = nc.dram_tensor("y", [P, NCH], I32, kind="ExternalInput")
    w_d = nc.dram_tensor("w", [C, D], F32, kind="ExternalInput")
    wt_d = nc.dram_tensor("wt", [D, C], F32, kind="ExternalInput")
    # res columns: 0 sums(c) 1 counts(c) 2 wsq(c) 3 g_rs(c) 4 o2(p) 5 og(p)
    res_d = nc.dram_tensor("res", [P, 8], F32, kind="ExternalOutput")

    NRHS = D + 2  # z_own | zsq | ones

    with tile.TileContext(nc) as tc:
        with (
            tc.tile_pool(name="cpool", bufs=1) as cpool,
            tc.tile_pool(name="sb", bufs=1) as sb,
            tc.tile_pool(name="rhsp", bufs=NCH) as rhsp,
            tc.tile_pool(name="scr", bufs=2) as scr,
            tc.tile_pool(name="psp", bufs=1, space="PSUM") as psp,
        ):
            # ---- y first: the gather-index chain is the critical path ----
            y_tile = sb.tile([P, NCH], I32)
            nc.sync.dma_start(out=y_tile[:], in_=y_d[:, :])
            # iota_ri[p, n] = (8p + n) * C  (row index base into z_flat)
            iota_ri = cpool.tile([P, NCH], I32)
            nc.gpsimd.iota(
                iota_ri[:], pattern=[[C, NCH]], base=0, channel_multiplier=C * NCH
            )
            row_idx = sb.tile([P, NCH], I32)
            nc.vector.tensor_add(out=row_idx[:], in0=y_tile[:], in1=iota_ri[:])

            # ---- launch all gathers as early as possible ----
            psum_acc = psp.tile([P, NRHS], F32)
            rhss = []
            for n in range(NCH):
                rhs = rhsp.tile([P, NRHS], F32, tag="rhs", name=f"rhs{n}")
                nc.gpsimd.indirect_dma_start(
                    out=rhs[:, 0:D],
                    out_offset=None,
                    in_=z_d[:, :],
                    in_offset=bass.IndirectOffsetOnAxis(
                        ap=row_idx[:, n : n + 1], axis=0
                    ),
                )
                rhss.append(rhs)

            # ---- one-hot ingredients (DVE, overlaps the gathers) ----
            iota_c = cpool.tile([P, C], F32)
            nc.gpsimd.iota(
                iota_c[:],
                pattern=[[1, C]],
                base=0,
                channel_multiplier=0,
                allow_small_or_imprecise_dtypes=True,
            )
            y_f = sb.tile([P, NCH], F32)
            nc.vector.tensor_copy(out=y_f[:], in_=y_tile[:])
            onehot = sb.tile([P, NCH * C], F32)
            for n in range(NCH):
                nc.vector.tensor_scalar(
                    out=onehot[:, n * C : (n + 1) * C],
                    in0=iota_c[:],
                    scalar1=y_f[:, n : n + 1],
                    scalar2=None,
                    op0=OP.is_equal,
                )

            # ---- per-chunk: zsq column, ones column, accumulate matmul ----
            for n in range(NCH):
                rhs = rhss[n]
                z_sq_scr = scr.tile([P, D], F32, tag="zsq", name=f"zsq{n}")
                nc.vector.scalar_tensor_tensor(
                    out=z_sq_scr[:],
                    in0=rhs[:, 0:D],
                    scalar=1.0,
                    op0=OP.mult,
                    in1=rhs[:, 0:D],
                    op1=OP.mult,
                    accum_out=rhs[:, D : D + 1],
                )
                nc.vector.memset(rhs[:, D + 1 : D + 2], 1.0)
                nc.tensor.matmul(
                    out=psum_acc[:C, :],
                    lhsT=onehot[:, n * C : (n + 1) * C],
                    rhs=rhs[:, :],
                    start=(n == 0),
                    stop=(n == NCH - 1),
                )

            # ---- o: per-partition reductions for loss_cls ----
            o_tile = sb.tile([P, NCH * C], F32)
            nc.sync.dma_start(out=o_tile[:], in_=o_d[:, :])
            opart = sb.tile([P, 2], F32)
            o_sq_scr = scr.tile([P, NCH * C], F32, tag="osq")
            nc.scalar.activation(
                out=o_sq_scr[:], in_=o_tile[:], func=AF.Square,
                accum_out=opart[:, 0:1],
            )
            og_scr = scr.tile([P, NCH * C], F32, tag="og")
            nc.vector.scalar_tensor_tensor(
                out=og_scr[:],
                in0=o_tile[:],
                scalar=1.0,
                op0=OP.mult,
                in1=onehot[:],
                op1=OP.mult,
                accum_out=opart[:, 1:2],
            )

            # ---- W: wsq, and pairwise-distance row sums for loss_dist ----
            w_tile = sb.tile([P, D], F32)
            nc.vector.memset(w_tile[:], 0.0)
            nc.sync.dma_start(out=w_tile[:C, :], in_=w_d[:, :])
            wsq = sb.tile([P, 1], F32)
            w_sq_scr = scr.tile([P, D], F32, tag="zsq")
            nc.scalar.activation(
                out=w_sq_scr[:], in_=w_tile[:], func=AF.Square,
                accum_out=wsq[:, 0:1],
            )
            wt_sb = sb.tile([P, C], F32)
            nc.sync.dma_start(out=wt_sb[:], in_=wt_d[:, :])
            psum_g = psp.tile([P, C], F32)
            nc.tensor.matmul(
                out=psum_g[:C, :],
                lhsT=wt_sb[:, :],
                rhs=wt_sb[:, :],
                start=True,
                stop=True,
            )
            g_rs = sb.tile([P, 1], F32)
            nc.vector.tensor_reduce(
                out=g_rs[:C, 0:1],
                in_=psum_g[:C, :],
                axis=mybir.AxisListType.X,
                op=OP.add,
            )

            # ---- finalize per-class sums and emit result tile ----
            # sums_c = zsq_class_c - 2 * <M_c, w_c> + count_c * wsq_c
            fin_scr = scr.tile([P, D], F32, tag="zsq")
            row_dot = sb.tile([P, 1], F32)
            nc.vector.scalar_tensor_tensor(
                out=fin_scr[:C, :],
                in0=psum_acc[:C, 0:D],
                scalar=1.0,
                op0=OP.mult,
                in1=w_tile[:C, :],
                op1=OP.mult,
                accum_out=row_dot[:C, 0:1],
            )
            res = sb.tile([P, 8], F32)
            nc.vector.memset(res[:], 0.0)
            t_cw = sb.tile([P, 1], F32)
            nc.vector.tensor_tensor(
                out=t_cw[:C, :],
                in0=psum_acc[:C, D + 1 : D + 2],
                in1=wsq[:C, :],
                op=OP.mult,
            )
            s1 = sb.tile([P, 1], F32)
            nc.vector.scalar_tensor_tensor(
                out=s1[:C, :],
                in0=row_dot[:C, :],
                scalar=-2.0,
                op0=OP.mult,
                in1=psum_acc[:C, D : D + 1],
                op1=OP.add,
            )
            nc.vector.tensor_add(out=res[:C, 0:1], in0=s1[:C, :], in1=t_cw[:C, :])
            nc.vector.tensor_copy(out=res[:C, 1:2], in_=psum_acc[:C, D + 1 : D + 2])
            nc.vector.tensor_copy(out=res[:C, 2:3], in_=wsq[:C, :])
            nc.vector.tensor_copy(out=res[:C, 3:4], in_=g_rs[:C, :])
            nc.vector.tensor_copy(out=res[:, 4:5], in_=opart[:, 0:1])
            nc.vector.tensor_copy(out=res[:, 5:6], in_=opart[:, 1:2])
            nc.sync.dma_start(out=res_d[:, :], in_=res[:])

    nc.compile()
    return nc


_NC = None


def _get_module():
    global _NC
    if _NC is None:
        _NC = build_module()
    return _NC


def make_in_maps(z, kernel_weight, o, y):
    z = np.asarray(z, dtype=np.float32)
    o = np.asarray(o, dtype=np.float32)
    w = np.ascontiguousarray(np.asarray(kernel_weight, dtype=np.float32))
    wt = np.ascontiguousarray(w.T)
    y32 = np.asarray(y).astype(np.int32)
    in_maps = []
    for ci in range(NCORES):
        sl = slice(ci * BL, (ci + 1) * BL)
        in_maps.append(
            {
                "z": np.ascontiguousarray(z[sl]).reshape(BL * C, D),
                "o": np.ascontiguousarray(o[sl]).reshape(P, NCH * C),
                "y": np.ascontiguousarray(y32[sl]).reshape(P, NCH),
                "w": w,
                "wt": wt,
            }
        )
    return in_maps


def combine(res_list):
    R = np.stack([np.asarray(r["res"], dtype=np.float64) for r in res_list])
    sums = R[:, :C, 0].sum(axis=0)
    counts = R[:, :C, 1].sum(axis=0)
    o2 = R[:, :, 4].sum()
    og = R[:, :, 5].sum()
    wsq = R[0, :C, 2]
    g_rs = R[0, :C, 3]

    loss_cls = (o2 - 2.0 * og + B) / (B * C)
    loss_close = float(np.sum(sums / (counts * D)))
    sw = wsq.sum()
    S = (C * wsq + sw - 2.0 * g_rs) / ((C - 1) * D)
    loss_dist = float(np.sum(np.maximum(MARGIN - S, 0.0)) / C)
    total = loss_cls + loss_close + loss_dist
    return (
        np.float32(total),
        np.float32(loss_cls),
        np.float32(loss_close),
        np.float32(loss_dist),
    )


def run_sharded(z, kernel_weight, o, y, **kwargs):
    nc = _get_module()
    in_maps = make_in_maps(z, kernel_weight, o, y)
    return run_bass_kernel_spmd(nc, in_maps, core_ids=list(range(NCORES)), **kwargs)


def kernel(z, kernel_weight, o, y):
    return combine(run_sharded(z, kernel_weight, o, y).results)
